# revision 5
# baseline (speedup 1.0000x reference)
"""DNADecoder TRN2 kernel v2: 3-core serial ring, SBUF-resident weights,
remote_dma handoffs (no DRAM mailboxes, no polling, no out_w streaming).

Core 0: emb gather + layers 0-1  -> bcast x1 to core 1
Core 1: layers 2-3               -> bcast x3 to core 2
Core 2: resident out_w logits + argmax + softmax + probs; token -> core 0.

Host precomputes: cross-attention collapse (exact: memory rows identical
across T so attention is uniform), self-attention fusion (Wv@Wo), weight
block packing.  All fp32, feature-major ("xT") activations.

wblk_sb is the universal per-role weight store: roles 0/1 hold their two
layers' blocks; role 2's in_map packs out_w as 32 [128,512] moving slices
plus out_b on partition 0.
"""

import numpy as np

B, T_FULL, P_IN, D, F, V, L, H = 64, 128, 1024, 512, 2048, 4096, 4, 8
NJ = D // 128             # 4 feature blocks of x
XC = NJ * B               # 256 cols of xT layout [128, 4*64]
NM1 = F // 128            # 16
SEQ0 = [(0, "sa"), (0, "w1"), (0, "w2"), (1, "sa"), (1, "w1"), (1, "w2")]
SEQ1 = [(2, "sa"), (2, "w1"), (2, "w2"), (3, "sa"), (3, "w1"), (3, "w2")]
NTILES = 288              # weight tiles per role (roles 0/1)
WCOLS = NTILES * 128      # 36864 cols = 147KB/partition
NCHUNK = 8                # out_w chunks of [512, 512] (4 per head core)
VH = V // 2               # vocab half per head core
NCH = 4                   # chunks per head core
OUTB_COL = 4 * NCH * 512  # bias row offset in head wblk (8192)
EPS = 1e-5

# remote ring: role r sends to DEST_TPB[r] (XOR-delta computed from tpb ids)
#   c0 -> c1 (x1), c1 -> c2 (x3), c2 -> c0 (token)


def _shapes(mat):
    return {"sa": (4, 4), "w1": (4, 16), "w2": (16, 4)}[mat]


def build_program(T, lite=False):
    import concourse.bass as bass
    import concourse.mybir as mybir
    from concourse import bacc
    from concourse.bass import ds

    f32 = mybir.dt.float32
    i32 = mybir.dt.int32
    u32 = mybir.dt.uint32
    A = mybir.AluOpType
    AF = mybir.ActivationFunctionType

    nc = bacc.Bacc("TRN2", target_bir_lowering=False)

    # ---- I/O ----
    inp = {}
    for name, shape, dt in [
        ("wblk", [128, WCOLS], f32), ("pext", [128, 4 * min(T, 128)], f32),
        ("lnp", [128, 96], f32), ("bsa", [128, 16], f32),
        ("b1x", [128, 64], f32), ("b2x", [128, 16], f32),
        ("caxt", [128, 1024], f32), ("consts", [128, 130], f32),
        ("crow", [1, 128], f32),
        ("tok0", [128, 1], i32),
        ("emb_hbm", [V, D], f32),
    ]:
        inp[name] = nc.declare_dram_parameter(name, shape, dt, isOutput=False)
    probs_out = nc.declare_dram_parameter(
        "probs_out", [B, 1 if lite else T, V], f32, isOutput=True)

    # lite (timing) builds keep an 8-slot probs ring so T can exceed DRAM
    # page limits; full builds store all T steps.
    TP = min(T, 128)            # pe table period
    PR = 8 if lite else T       # probs ring size
    probs_int = nc.dram_tensor("probs_int", [B, PR, VH], f32)
    emb_int = nc.dram_tensor("emb_int", [V, D], f32)

    ctxs = []

    def sb(name, shape, dt=f32):
        cm = nc.sbuf_tensor(name, shape, dt)
        h = cm.__enter__()
        ctxs.append(cm)
        return h

    def psum(name, shape):
        cm = nc.psum_tensor(name, shape, f32)
        h = cm.__enter__()
        ctxs.append(cm)
        return h

    wblk_sb = sb("wblk_sb", [128, WCOLS])
    pext_sb = sb("pext_sb", [128, 4 * TP])
    lnp_sb = sb("lnp_sb", [128, 96])
    bsa_sb = sb("bsa_sb", [128, 16])
    b1x_sb = sb("b1x_sb", [128, 64])
    b2x_sb = sb("b2x_sb", [128, 16])
    caxt_sb = sb("caxt_sb", [128, 1024])
    consts_sb = sb("consts_sb", [128, 130])
    crow_sb = sb("crow_sb", [1, 128])
    E_sb = sb("E_sb", [64, 512])
    pe_cur = sb("pe_cur", [128, 4])
    x_sb = sb("x_sb", [128, XC])          # working activation + rx buffer
    y_sb = sb("y_sb", [128, XC])
    tmp_sb = sb("tmp_sb", [128, XC])
    sq_sb = sb("sq_sb", [128, XC])
    h_sb = sb("h_sb", [128, 1024])
    stats_sb = sb("stats_sb", [1, 128])
    msq_sb = sb("msq_sb", [1, 64])
    var_sb = sb("var_sb", [1, 64])
    rstd_sb = sb("rstd_sb", [1, 64])
    logits_sb = sb("logits_sb", [64, VH])
    mx_sb = sb("mx_sb", [64, 8])
    idx_sb = sb("idx_sb", [64, 8], u32)
    tok_pad = sb("tok_pad", [128, 1], i32)  # c2 tx / c0 rx (token)
    negmx_sb = sb("negmx_sb", [64, 1])
    sums_sb = sb("sums_sb", [64, 1])
    sums2_sb = sb("sums2_sb", [64, 1])
    rec_sb = sb("rec_sb", [64, 1])
    mxp = sb("mxp", [128, 1])      # tx: local max (rows 0:64)
    idxfp = sb("idxfp", [128, 1])  # tx: local argmax as f32
    sumsp = sb("sumsp", [128, 1])  # tx: local expsum
    mxq = sb("mxq", [128, 1])      # rx: peer max
    idxq = sb("idxq", [128, 1])    # rx: peer argmax f32
    sumsq = sb("sumsq", [128, 1])  # rx: peer expsum
    tokf_sb = sb("tokf_sb", [64, 1])
    gmax_sb = sb("gmax_sb", [64, 1])

    ps_big = psum("ps_big", [128, 1024])
    ps_tr = psum("ps_tr", [128, 256])
    ps_misc = psum("ps_misc", [128, 512])
    ps_log = psum("ps_log", [64, 1024])

    ident = consts_sb[:, 0:128]
    sc512 = consts_sb[:, 128:129]
    onesr = crow_sb[0:1, 0:128]
    ones64 = crow_sb[0:1, 0:64]

    SEMS = ["s_g", "s_p", "s_v", "s_a", "s_dg",
            "s_rx", "s_tk", "s_loc", "s_prep", "s_pd", "s_hx"]
    sem_h = {}
    for s in SEMS:
        cm = nc.semaphore(s)
        sem_h[s] = cm.__enter__()
        ctxs.append(cm)

    # ---------------- schedule builder ----------------
    def build_sched(role, t, dyn, geng):
        """t: python int for peeled step 0, else None (loop body).
        dyn: dict of registers (only when walking engine that owns them)."""
        ops = []

        def op(eng, emit, waits=(), signals=()):
            ops.append((eng, emit, tuple(waits), tuple(signals)))

        def gdma(out, in_, indirect=None):
            g = geng
            if indirect is None:
                ins = g.dma_start(out=out, in_=in_)
            else:
                ins = g.indirect_dma_start(
                    out=out, out_offset=None, in_=in_,
                    in_offset=bass.IndirectOffsetOnAxis(ap=indirect, axis=0),
                )
            ins.then_inc(sem_h["s_dg"], 16)
            g.reg_add(dyn["cnt"], dyn["cnt"], 16)
            return ins

        def gwait():
            geng.wait_ge(sem_h["s_dg"], dyn["cnt"])

        def bcast(in_ap, out_ap, slot, dtpb, sem_name="s_rx"):
            # one remote bcast: single real dest at `slot`; lanes(slot,slot+8)
            g = geng
            rdests = [None] * 8
            rdests[slot] = (0, dtpb)
            ins = g.remote_dma_broadcast(
                out_ap=out_ap, in_ap=in_ap,
                remote_sem=sem_h[sem_name],
                local_sem=sem_h["s_loc"], rdests=rdests)
            return ins

        def emit_ln(lidx, ln_i, ysem):
            gb = lambda gb_i, j: lnp_sb[:, lidx * 24 + ln_i * 8 + gb_i * 4 + j:
                                        lidx * 24 + ln_i * 8 + gb_i * 4 + j + 1]

            def esq(e):
                return e.activation(out=sq_sb[:, :], in_=y_sb[:, :], func=AF.Square)
            op("a", esq, waits=[ysem], signals=[("s_a", 1)])

            def esums(e):
                ii = None
                for j in range(NJ):
                    ii = e.matmul(out=ps_misc[0:1, 0:64], lhsT=sc512,
                                  rhs=y_sb[:, j * 64:(j + 1) * 64],
                                  start=(j == 0), stop=(j == NJ - 1))
                for j in range(NJ):
                    ii = e.matmul(out=ps_misc[0:1, 64:128], lhsT=sc512,
                                  rhs=sq_sb[:, j * 64:(j + 1) * 64],
                                  start=(j == 0), stop=(j == NJ - 1))
                return ii
            op("p", esums, waits=["s_a"], signals=[("s_p", 1)])

            def evar(e):
                e.tensor_copy(out=stats_sb[:, :], in_=ps_misc[0:1, 0:128])
                e.drain()
                e.tensor_tensor(out=msq_sb[:, :], in0=stats_sb[0:1, 0:64],
                                in1=stats_sb[0:1, 0:64], op=A.mult)
                e.drain()
                return e.tensor_tensor(out=var_sb[:, :], in0=stats_sb[0:1, 64:128],
                                       in1=msq_sb[:, :], op=A.subtract)
            op("v", evar, waits=["s_p"], signals=[("s_v", 1)])

            op("a", lambda e: e.activation(out=var_sb[:, :], in_=var_sb[:, :],
                                           func=AF.Sqrt, bias=consts_sb[0:1, 129:130]),
               waits=["s_v"], signals=[("s_a", 1)])
            op("v", lambda e: e.reciprocal(out=rstd_sb[:, :], in_=var_sb[:, :]),
               waits=["s_a"], signals=[("s_v", 1)])

            def ebc(e):
                e.matmul(out=ps_misc[:, 128:192], lhsT=onesr,
                         rhs=stats_sb[0:1, 0:64], start=True, stop=True)
                return e.matmul(out=ps_misc[:, 192:256], lhsT=onesr,
                                rhs=rstd_sb[0:1, 0:64], start=True, stop=True)
            op("p", ebc, waits=["s_v"], signals=[("s_p", 1)])

            def enrm(e):
                ii = None
                for j in range(NJ):
                    ii = e.tensor_tensor(out=tmp_sb[:, j * 64:(j + 1) * 64],
                                         in0=y_sb[:, j * 64:(j + 1) * 64],
                                         in1=ps_misc[:, 128:192], op=A.subtract)
                e.drain()
                for j in range(NJ):
                    ii = e.tensor_tensor(out=tmp_sb[:, j * 64:(j + 1) * 64],
                                         in0=tmp_sb[:, j * 64:(j + 1) * 64],
                                         in1=ps_misc[:, 192:256], op=A.mult)
                return ii
            op("v", enrm, waits=["s_p"], signals=[("s_v", 1)])

            def eaff(e):
                e.drain()
                ii = None
                for j in range(NJ):
                    ii = e.tensor_scalar(out=x_sb[:, j * 64:(j + 1) * 64],
                                         in0=tmp_sb[:, j * 64:(j + 1) * 64],
                                         scalar1=gb(0, j), scalar2=gb(1, j),
                                         op0=A.mult, op1=A.add)
                return ii
            op("v", eaff, waits=["s_p"], signals=[("s_v", 1)])

        def wcol(mm_i, m, k):
            base = 0
            seq = SEQ0 if role == 0 else SEQ1
            for i in range(mm_i):
                nk, nm = _shapes(seq[i][1])
                base += nk * nm
            nk, nm = _shapes(seq[mm_i][1])
            return (base + m * nk + k) * 128

        def emit_wsmm(mm_i, mat, xsem, dst, dcol0, extra_wait=None):
            nk, nm = _shapes(mat)
            src = h_sb if mat == "w2" else x_sb

            def emm(e):
                if extra_wait is not None:
                    extra_wait(e)
                ii = None
                for m in range(nm):
                    for k in range(nk):
                        ii = e.matmul(
                            out=dst[:, dcol0 + m * 64:dcol0 + (m + 1) * 64],
                            lhsT=wblk_sb[:, wcol(mm_i, m, k):
                                         wcol(mm_i, m, k) + 128],
                            rhs=src[:, k * 64:(k + 1) * 64],
                            start=(k == 0), stop=(k == nk - 1))
                return ii
            op("p", emm, waits=[xsem] if xsem else [], signals=[("s_p", 1)])

        def emit_layer(lidx, mm_base, xsem, extra_wait=None):
            emit_wsmm(mm_base + 0, "sa", xsem, ps_big, 0, extra_wait)

            def eres_sa(e):
                e.drain()
                return e.tensor_tensor(out=y_sb[:, :], in0=x_sb[:, :],
                                       in1=ps_big[:, 0:XC], op=A.add)
            op("v", eres_sa, waits=["s_p"], signals=[("s_v", 1)])

            def ebias_sa(e):
                e.drain()
                ii = None
                for j in range(NJ):
                    ii = e.tensor_scalar(out=y_sb[:, j * 64:(j + 1) * 64],
                                         in0=y_sb[:, j * 64:(j + 1) * 64],
                                         scalar1=bsa_sb[:, lidx * 4 + j:
                                                        lidx * 4 + j + 1],
                                         scalar2=None, op0=A.add)
                return ii
            op("v", ebias_sa, waits=[], signals=[("s_v", 1)])
            emit_ln(lidx, 0, "s_v")

            def eca(e):
                e.drain()
                return e.tensor_tensor(out=y_sb[:, :], in0=x_sb[:, :],
                                       in1=caxt_sb[:, lidx * 256:(lidx + 1) * 256],
                                       op=A.add)
            op("v", eca, waits=[], signals=[("s_v", 1)])
            emit_ln(lidx, 1, "s_v")

            emit_wsmm(mm_base + 1, "w1", "s_v", ps_big, 0)

            def erelu(e):
                ii = None
                for m in range(NM1):
                    ii = e.activation(out=h_sb[:, m * 64:(m + 1) * 64],
                                      in_=ps_big[:, m * 64:(m + 1) * 64],
                                      func=AF.Relu,
                                      bias=b1x_sb[:, lidx * 16 + m:lidx * 16 + m + 1])
                return ii
            op("a", erelu, waits=["s_p"], signals=[("s_a", 1)])

            emit_wsmm(mm_base + 2, "w2", "s_a", ps_big, 0)

            def eres_f(e):
                e.drain()
                return e.tensor_tensor(out=y_sb[:, :], in0=x_sb[:, :],
                                       in1=ps_big[:, 0:XC], op=A.add)
            op("v", eres_f, waits=["s_p"], signals=[("s_v", 1)])

            def ebias_f(e):
                e.drain()
                ii = None
                for j in range(NJ):
                    ii = e.tensor_scalar(out=y_sb[:, j * 64:(j + 1) * 64],
                                         in0=y_sb[:, j * 64:(j + 1) * 64],
                                         scalar1=b2x_sb[:, lidx * 4 + j:
                                                        lidx * 4 + j + 1],
                                         scalar2=None, op0=A.add)
                return ii
            op("v", ebias_f, waits=[], signals=[("s_v", 1)])
            emit_ln(lidx, 2, "s_v")

        def emit_xsend(dtpbs):
            # send x_sb (128KB) as 4 slices per dest, slots spread, then trigger
            def esend(e):
                nprep = 0
                for di, dtpb in enumerate(dtpbs):
                    for j in range(NJ):
                        ins = bcast(x_sb[:, j * 64:(j + 1) * 64],
                                    x_sb[:, j * 64:(j + 1) * 64],
                                    di * 4 + j, dtpb)
                        ins.then_inc(sem_h["s_prep"], 1)
                        nprep += 1
                e.reg_add(dyn["prep"], dyn["prep"], nprep)
                e.wait_ge(sem_h["s_prep"], dyn["prep"])
                return e.trigger_dma(nprep)
            op("g", esend, waits=["s_v"], signals=())

        # ================= role 0: emb + L0 + L1 =================
        if role == 0:
            def eg_in(e):
                if t is None:
                    # wait token from c2: s_tk >= 2*t
                    e.reg_add(dyn["tok"], dyn["tok"], 2)
                    e.wait_ge(sem_h["s_tk"], dyn["tok"])
                gdma(E_sb[:, :], emb_int[:, :], indirect=tok_pad[0:64, 0:1])
                gdma(pe_cur[:, :], pext_sb[:, ds(dyn["t4"], 4)]
                     if t is None else pext_sb[:, 4 * t:4 * t + 4])
                gwait()
                return e.nop()
            op("g", eg_in, waits=["s_a", "s_p", "s_v"], signals=[("s_g", 1)])

            def etr(e):
                ii = None
                for j in range(NJ):
                    ii = e.transpose(out=ps_tr[:, j * 64:(j + 1) * 64],
                                     in_=E_sb[:, j * 128:(j + 1) * 128],
                                     identity=ident[0:64, 0:64])
                return ii
            op("p", etr, waits=["s_g"], signals=[("s_p", 1)])

            def ex0(e):
                e.drain()
                # guard: previous step's x sends completed (s_loc >= 64*t)
                if t is None:
                    e.reg_add(dyn["vloc"], dyn["vloc"], 64)
                    e.wait_ge(sem_h["s_loc"], dyn["vloc"])
                ii = None
                for j in range(NJ):
                    ii = e.tensor_scalar(out=x_sb[:, j * 64:(j + 1) * 64],
                                         in0=ps_tr[:, j * 64:(j + 1) * 64],
                                         scalar1=pe_cur[:, j:j + 1], scalar2=None,
                                         op0=A.add)
                return ii
            op("v", ex0, waits=["s_p"], signals=[("s_v", 1)])

            emit_layer(0, 0, "s_v")
            emit_layer(1, 3, "s_v")
            emit_xsend([1])   # c0 -> c1 : dtpb = 0^1

        # ================= role 1: L2 + L3 =================
        elif role == 1:
            # first consumer (p) waits arrivals: s_rx >= 8*(t+1)
            def rxwait(e):
                e.reg_add(dyn["rx"], dyn["rx"], 8)
                e.wait_ge(sem_h["s_rx"], dyn["rx"])
            emit_layer(2, 0, None, extra_wait=rxwait)
            emit_layer(3, 3, "s_v")
            emit_xsend([3, 2])   # c1 -> c2 (1^2=3) and c1 -> c3 (1^3=2)

        # ============ roles 2/3: split head (vocab halves) ============
        else:
            cons = []
            for n in range(NCH):
                def econs(e, n=n):
                    if n == 0:
                        e.reg_add(dyn["rx"], dyn["rx"], 8)
                        e.wait_ge(sem_h["s_rx"], dyn["rx"])
                    sl = (n % 2)
                    for k in range(4):
                        e.matmul(out=ps_log[:, sl * 512:(sl + 1) * 512],
                                 lhsT=x_sb[:, k * 64:(k + 1) * 64],
                                 rhs=wblk_sb[:, (4 * n + k) * 512:
                                             (4 * n + k + 1) * 512],
                                 start=(k == 0), stop=False)
                    return e.matmul(out=ps_log[:, sl * 512:(sl + 1) * 512],
                                    lhsT=ones64,
                                    rhs=wblk_sb[0:1, OUTB_COL + n * 512:
                                                OUTB_COL + (n + 1) * 512],
                                    start=False, stop=True)
                cons.append(econs)

            for n in range(NCH):
                op("p", cons[n], waits=(["s_a"] if n >= 2 else []),
                   signals=[("s_p", 1)])

                def ecp(e, n=n):
                    return e.activation(out=logits_sb[:, n * 512:(n + 1) * 512],
                                        in_=ps_log[:, (n % 2) * 512:
                                                   ((n % 2) + 1) * 512],
                                        func=AF.Copy)
                op("a", ecp, waits=["s_p"] + (["s_pd"] if n == 0 else []),
                   signals=[("s_a", 1)])

            def emax(e):
                e.max(out=mx_sb[:, :], in_=logits_sb[:, :])
                e.drain()
                e.max_index(out=idx_sb[:, :], in_max=mx_sb[:, :],
                            in_values=logits_sb[:, :])
                e.drain()
                if t is None:
                    e.reg_add(dyn["vloc"], dyn["vloc"],
                              48 if role == 2 else 48)
                    e.wait_ge(sem_h["s_loc"], dyn["vloc"])
                e.tensor_copy(out=mxp[0:64, :], in_=mx_sb[:, 0:1])
                e.drain()
                return e.tensor_copy(out=idxfp[0:64, :], in_=idx_sb[:, 0:1])
            op("v", emax, waits=["s_a"], signals=[("s_v", 1)])

            def ehx1(e):
                i1 = bcast(mxp[:, :], mxq[:, :], 0, 1, "s_hx")
                i1.then_inc(sem_h["s_prep"], 1)
                i2 = bcast(idxfp[:, :], idxq[:, :], 1, 1, "s_hx")
                i2.then_inc(sem_h["s_prep"], 1)
                e.reg_add(dyn["prep"], dyn["prep"], 2)
                e.wait_ge(sem_h["s_prep"], dyn["prep"])
                return e.trigger_dma(2)
            op("g", ehx1, waits=["s_v"], signals=())

            def emerge(e):
                # wait peer (mx,idx) arrival: +4/step on s_hx
                e.reg_add(dyn["hx"], dyn["hx"], 4)
                e.wait_ge(sem_h["s_hx"], dyn["hx"])
                e.tensor_tensor(out=gmax_sb[:, :], in0=mx_sb[:, 0:1],
                                in1=mxq[0:64, :], op=A.max)
                e.drain()
                e.tensor_scalar(out=negmx_sb[:, :], in0=gmax_sb[:, :],
                                scalar1=-1.0, scalar2=None, op0=A.mult)
                if role == 2:
                    e.drain()
                    # token = mx_loc >= mx_peer ? idx_loc : idx_peer + 2048
                    e.tensor_scalar(out=tokf_sb[:, :], in0=idxq[0:64, :],
                                    scalar1=2048.0, scalar2=None, op0=A.add)
                    e.drain()
                    e.tensor_tensor(out=idxq[0:64, :], in0=idxfp[0:64, :],
                                    in1=tokf_sb[:, :], op=A.subtract)
                    e.drain()
                    e.tensor_tensor(out=mxq[0:64, :], in0=mx_sb[:, 0:1],
                                    in1=mxq[0:64, :], op=A.is_ge)
                    e.drain()
                    e.tensor_tensor(out=idxq[0:64, :], in0=idxq[0:64, :],
                                    in1=mxq[0:64, :], op=A.mult)
                    e.drain()
                    e.tensor_tensor(out=tokf_sb[:, :], in0=tokf_sb[:, :],
                                    in1=idxq[0:64, :], op=A.add)
                    e.drain()
                    ii = e.tensor_copy(out=tok_pad[0:64, :], in_=tokf_sb[:, :])
                else:
                    ii = e.nop()
                return ii
            op("v", emerge, waits=[], signals=[("s_v", 1)])

            if role == 2:
                def eg_tok(e):
                    ins = bcast(tok_pad[:, :], tok_pad[:, :], 2, 2, "s_tk")
                    ins.then_inc(sem_h["s_prep"], 1)
                    e.reg_add(dyn["prep"], dyn["prep"], 1)
                    e.wait_ge(sem_h["s_prep"], dyn["prep"])
                    return e.trigger_dma(1)
                op("g", eg_tok, waits=["s_v"], signals=())

            def eexp(e):
                return e.activation(out=logits_sb[:, :], in_=logits_sb[:, :],
                                    func=AF.Exp, bias=negmx_sb[:, 0:1],
                                    accum_out=sums_sb[:, :])
            op("a", eexp, waits=["s_v", "s_pd"], signals=[("s_a", 1)])

            def esump(e):
                e.drain()
                return e.tensor_copy(out=sumsp[0:64, :], in_=sums_sb[:, :])
            op("v", esump, waits=["s_a"], signals=[("s_v", 1)])

            def ehx2(e):
                ins = bcast(sumsp[:, :], sumsq[:, :], 3, 1, "s_hx")
                ins.then_inc(sem_h["s_prep"], 1)
                e.reg_add(dyn["prep"], dyn["prep"], 1)
                e.wait_ge(sem_h["s_prep"], dyn["prep"])
                return e.trigger_dma(1)
            op("g", ehx2, waits=["s_v"], signals=())

            def egsum(e):
                # wait peer sum arrival: +2/step on s_hx
                e.reg_add(dyn["hx"], dyn["hx"], 2)
                e.wait_ge(sem_h["s_hx"], dyn["hx"])
                e.tensor_tensor(out=sums2_sb[:, :], in0=sums_sb[:, :],
                                in1=sumsq[0:64, :], op=A.add)
                e.drain()
                return e.reciprocal(out=rec_sb[:, :], in_=sums2_sb[:, :])
            op("v", egsum, waits=[], signals=[("s_v", 1)])

            def escale(e):
                return e.activation(out=logits_sb[:, :], in_=logits_sb[:, :],
                                    func=AF.Copy, scale=rec_sb[:, 0:1])
            op("a", escale, waits=["s_v", "s_a"], signals=[("s_a", 1)])

            def eg_probs(e):
                ins = e.dma_start(out=probs_int[:, dyn["tm"] if t is None else (t % PR), :],
                                  in_=logits_sb[:, :])
                ins.then_inc(sem_h["s_pd"], 16)
                return e.nop()
            op("g", eg_probs, waits=["s_a"], signals=[("s_pd", 16)])

        return ops

    # ---------------- walker ----------------
    class Walker:
        def __init__(self, eng_name, eng):
            self.en = eng_name
            self.e = eng
            self.counts = {s: 0 for s in SEMS}
            self.last = {}
            self.regs = {}

        def prealloc(self, scheds):
            need = set()
            for sched in scheds:
                for eng, _, waits, _ in sched:
                    if eng == self.en:
                        for s in waits:
                            need.add(s)
            for s in sorted(need):
                r = self.e.alloc_register(f"thr_{self.en}_{s}")
                self.e.reg_mov(r, 0)
                self.regs[s] = r
                self.last[s] = 0

        def walk(self, sched):
            for eng, fn, waits, signals in sched:
                if eng == self.en:
                    for s in waits:
                        delta = self.counts[s] - self.last[s]
                        if delta > 0:
                            self.e.reg_add(self.regs[s], self.regs[s], delta)
                            self.last[s] = self.counts[s]
                        self.e.wait_ge(sem_h[s], self.regs[s])
                    ins = fn(self.e)
                    first = True
                    for s, amt in signals:
                        if s == "s_pd":
                            continue
                        if first:
                            ins.then_inc(sem_h[s], amt)
                            first = False
                        else:
                            self.e.nop().then_inc(sem_h[s], amt)
                for s, amt in signals:
                    self.counts[s] += amt

        def flush(self):
            for s, r in self.regs.items():
                delta = self.counts[s] - self.last[s]
                if delta > 0:
                    self.e.reg_add(r, r, delta)
                    self.last[s] = self.counts[s]

    def trace_engine(eng_name, eng):
        pid = eng.partition_id()
        role_r = eng.alloc_register(f"role_{eng_name}")
        eng.reg_alu(role_r, pid, 0, A.add)

        cnt0 = None
        if eng_name == "g":
            cnt0 = eng.alloc_register("cnt_init")
            eng.reg_mov(cnt0, 0)
            c = 0
            for dst, src in [
                (wblk_sb[:, :], inp["wblk"][:, :]),
                (pext_sb[:, :], inp["pext"][:, :]),
                (lnp_sb[:, :], inp["lnp"][:, :]),
                (bsa_sb[:, :], inp["bsa"][:, :]),
                (b1x_sb[:, :], inp["b1x"][:, :]),
                (b2x_sb[:, :], inp["b2x"][:, :]),
                (caxt_sb[:, :], inp["caxt"][:, :]),
                (consts_sb[:, :], inp["consts"][:, :]),
                (crow_sb[:, :], inp["crow"][:, :]),
                (tok_pad[:, :], inp["tok0"][:, :]),
                (emb_int[:, :], inp["emb_hbm"][:, :]),
            ]:
                eng.dma_start(out=dst, in_=src).then_inc(sem_h["s_dg"], 16)
                c += 16
            eng.reg_add(cnt0, cnt0, c)
            eng.wait_ge(sem_h["s_dg"], cnt0)
            init_sig = eng.nop()
            init_sig.then_inc(sem_h["s_g"], 1)

        w = Walker(eng_name, eng)

        # per-engine dyn registers (allocated once, reset per role branch)
        dyn_regs = {}
        for key in ["t", "t4", "tm", "rx", "tok", "vloc", "prep", "hx"]:
            dyn_regs[key] = eng.alloc_register(f"dyn_{eng_name}_{key}")

        for role in (0, 1, 2, 3):
            if role == 0:
                scheds = []
                for r in (0, 1, 2, 3):
                    dummy = {k: dyn_regs[k] for k in dyn_regs}
                    dummy["cnt"] = cnt0
                    scheds.append(build_sched(r, 0, dummy, eng if eng_name == "g" else None))
                w.prealloc(scheds)

            with eng.If_eq(role_r, role):
                for s, r in w.regs.items():
                    eng.reg_mov(r, 0)
                w.counts = {s: 0 for s in SEMS}
                w.counts["s_g"] = 1 if eng_name == "g" else 0
                # non-g engines never see the init s_g signal in their counts;
                # but they may wait on s_g -> account globally:
                w.counts["s_g"] = 1
                for s in w.last:
                    w.last[s] = 0
                for key in ["rx", "tok", "vloc", "prep", "hx"]:
                    eng.reg_mov(dyn_regs[key], 0)

                dyn = {k: dyn_regs[k] for k in dyn_regs}
                dyn["cnt"] = cnt0
                peel = build_sched(role, 0, dyn, eng if eng_name == "g" else None)
                w.walk(peel)
                w.flush()
                if T > 1:
                    with eng.Fori(1, T) as iv:
                        if eng_name == "g":
                            eng.reg_alu(dyn_regs["t"], iv, 1, A.mult)
                            eng.reg_alu(dyn_regs["t4"], iv, TP - 1,
                                        A.bitwise_and)
                            eng.reg_alu(dyn_regs["t4"], dyn_regs["t4"], 4,
                                        A.mult)
                            eng.reg_alu(dyn_regs["tm"], iv, PR - 1,
                                        A.bitwise_and)
                            dyn["t"] = eng.snap(dyn_regs["t"], min_val=0,
                                                max_val=T - 1)
                            dyn["t4"] = eng.snap(dyn_regs["t4"], min_val=0,
                                                 max_val=4 * (TP - 1))
                            dyn["tm"] = eng.snap(dyn_regs["tm"], min_val=0,
                                                 max_val=PR - 1)
                        body = build_sched(role, None, dyn,
                                           eng if eng_name == "g" else None)
                        w.walk(body)
                        w.flush()
                if eng_name == "g" and role in (2, 3):
                    eng.wait_ge(sem_h["s_dg"], cnt0)
                    eng.wait_ge(sem_h["s_pd"], 16 * T)
                    c = 0
                    vlo = 0 if role == 2 else VH
                    if lite:
                        eng.dma_start(out=probs_out[:, 0, vlo:vlo + VH],
                                      in_=probs_int[:, (T - 1) % PR, :]
                                      ).then_inc(sem_h["s_dg"], 16)
                        c += 16
                    else:
                        for b0 in range(0, B, 8):
                            eng.dma_start(out=probs_out[b0:b0 + 8, :, vlo:vlo + VH],
                                          in_=probs_int[b0:b0 + 8, :, :]
                                          ).then_inc(sem_h["s_dg"], 16)
                            c += 16
                    eng.reg_add(cnt0, cnt0, c)
                    eng.wait_ge(sem_h["s_dg"], cnt0)

    with nc.Block() as block:
        @block.gpsimd
        def _(g):
            trace_engine("g", g)

        @block.tensor
        def _(p):
            trace_engine("p", p)

        @block.vector
        def _(v):
            trace_engine("v", v)

        @block.scalar
        def _(a):
            trace_engine("a", a)

    nc.finalize()
    return nc


# ================= host side =================

def _pack(inputs, T):
    g = lambda k: np.asarray(inputs[k], np.float32)
    prot = g("protein_embeddings")
    tok0 = np.zeros((128, 1), np.int32)
    tok0[0:64, 0] = np.asarray(inputs["input_token"]).astype(np.int32)
    p1w, p1b = g("proj1_w"), g("proj1_b")
    p2w, p2b = g("proj2_w"), g("proj2_b")
    emb = g("emb")
    sa_w, sa_b = g("sa_w"), g("sa_b")
    ca_w, ca_b = g("ca_w"), g("ca_b")
    w1, b1 = g("ffn_w1"), g("ffn_b1")
    w2, b2 = g("ffn_w2"), g("ffn_b2")
    ln_g, ln_b = g("ln_g"), g("ln_b")
    out_w, out_b = g("out_w"), g("out_b")
    pe = g("pe")

    mem1 = np.maximum(prot @ p1w + p1b, 0.0) @ p2w + p2b
    ca_out = np.stack([(mem1 @ ca_w[l, 2] + ca_b[l, 2]) @ ca_w[l, 3] + ca_b[l, 3]
                       for l in range(L)])
    W_sa = np.stack([sa_w[l, 2] @ sa_w[l, 3] for l in range(L)])
    b_sa = np.stack([sa_b[l, 2] @ sa_w[l, 3] + sa_b[l, 3] for l in range(L)])

    def wmat(l, mat):
        return {"sa": W_sa[l], "w1": w1[l], "w2": w2[l]}[mat]

    def pack_wblk(seq):
        out = np.zeros((128, WCOLS), np.float32)
        cur = 0
        for (l, mat) in seq:
            W = wmat(l, mat)
            nk, nm = _shapes(mat)
            for m in range(nm):
                for k in range(nk):
                    out[:, cur * 128:(cur + 1) * 128] = \
                        W[128 * k:128 * (k + 1), 128 * m:128 * (m + 1)]
                    cur += 1
        return out

    def pack_wout(half):
        out = np.zeros((128, WCOLS), np.float32)
        for n in range(NCH):
            gn = half * NCH + n
            for k in range(4):
                out[:, (4 * n + k) * 512:(4 * n + k + 1) * 512] = \
                    out_w[128 * k:128 * (k + 1), 512 * gn:512 * (gn + 1)]
        out[0, OUTB_COL:OUTB_COL + VH] = out_b[half * VH:(half + 1) * VH]
        return out

    TP = min(T, 128)
    pext = np.zeros((128, 4 * TP), np.float32)
    for t in range(TP):
        for j in range(NJ):
            pext[:, 4 * t + j] = pe[t % pe.shape[0], 128 * j:128 * (j + 1)]

    lnp = np.zeros((128, 96), np.float32)
    for l in range(L):
        for ln in range(3):
            for j in range(NJ):
                lnp[:, l * 24 + ln * 8 + 0 + j] = ln_g[l, ln, 128 * j:128 * (j + 1)]
                lnp[:, l * 24 + ln * 8 + 4 + j] = ln_b[l, ln, 128 * j:128 * (j + 1)]

    bsa = np.zeros((128, 16), np.float32)
    b2x = np.zeros((128, 16), np.float32)
    b1x = np.zeros((128, 64), np.float32)
    caxt = np.zeros((128, 1024), np.float32)
    for l in range(L):
        for j in range(NJ):
            bsa[:, l * 4 + j] = b_sa[l, 128 * j:128 * (j + 1)]
            b2x[:, l * 4 + j] = b2[l, 128 * j:128 * (j + 1)]
            for b in range(B):
                caxt[:, l * 256 + j * 64 + b] = ca_out[l, b, 128 * j:128 * (j + 1)]
        for m in range(NM1):
            b1x[:, l * 16 + m] = b1[l, 128 * m:128 * (m + 1)]

    consts = np.zeros((128, 130), np.float32)
    consts[:, 0:128] = np.eye(128, dtype=np.float32)
    consts[:, 128] = 1.0 / 512.0
    consts[:, 129] = EPS
    crow = np.ones((1, 128), np.float32)

    common = dict(
        pext=pext, lnp=lnp, bsa=bsa, b1x=b1x, b2x=b2x, caxt=caxt,
        consts=consts, crow=crow, tok0=tok0, emb_hbm=emb,
    )
    zerow = np.zeros((128, WCOLS), np.float32)
    maps = []
    for c in range(8):
        if c == 0:
            maps.append(dict(common, wblk=pack_wblk(SEQ0)))
        elif c == 1:
            maps.append(dict(common, wblk=pack_wblk(SEQ1)))
        elif c == 2:
            maps.append(dict(common, wblk=pack_wout(0)))
        elif c == 3:
            maps.append(dict(common, wblk=pack_wout(1)))
        else:
            maps.append(dict(common, wblk=zerow))
    return maps


def kernel(**inputs):
    from concourse.bass_utils import run_bass_kernel_spmd

    T = T_FULL
    nc = build_program(T)
    in_maps = _pack(inputs, T)
    res = run_bass_kernel_spmd(nc, in_maps[:4], core_ids=[0, 1, 2, 3])
    out = np.asarray(res.results[2]["probs_out"], np.float32).copy()
    out[:, :, VH:] = np.asarray(res.results[3]["probs_out"], np.float32)[:, :, VH:]
    return out
